# revision 5
# baseline (speedup 1.0000x reference)
"""
CIN (Compressed Interaction Network) kernel for Trainium2, 8 NeuronCores.

Problem (hardcoded):
  x: [4096, 32, 64] fp32; w0: [128, 1024]; b0: [128]; w1: [128, 2048]; b1: [128]
  out: [4096, 192] = concat(relu(y0)[:, 64:], relu(y1)).sum(d)
  y0 = w0 @ vec(x (x) x) per (b, d) token; y1 = w1 @ vec(hidden (x) x).

Sharding: data parallel over batch, 512 samples/core, tokens t=(b,d), T=32768.

Per-core pipeline (pair = 2048 tokens):
  - L0 uses the polarization identity x_h*x_f = ((x_h+x_f)^2 - x_h^2 - x_f^2)/2:
    496 upper-triangle sum-channels + 32 square-channels = 528 channels
    (vs 1024 raw channels). Sum-channels u = A0 @ x are built on the PE as
    K=32 two-hot matmuls, 4-way tile_position concurrent (x is tiled 4x
    across partitions); ScalarE evacuates with func=Square; contraction
    uses host-folded weights. No broadcast DMA, no DVE muls for L0.
  - L1 keeps direct z = hx * xr: 4 groups get PE-built broadcast tiles
    (K=32 one-hot, 4-way via hidden duplicated to partitions 64:128), the
    other 12 are DMA-expanded from an HBM round-trip of the hidden rows
    (broadcast access pattern, 32x row replication on read).
  - z-muls on VectorE (13) + GpSimd (3); y evac ScalarE Relu+bias;
    d-sum reduce on VectorE.
"""

import sys

import numpy as np
import ml_dtypes

sys.path.insert(0, "/opt/trn_rl_repo")

B_FULL = 4096
N_CORES = 8
BS = B_FULL // N_CORES  # 512
F = 32
D = 64
T = BS * D  # 32768
PAIR = 2048  # tokens per pair (32 samples)
O = 128
H1 = 64
G1 = 16
CH0 = 528  # 496 pairs (h<f) + 32 squares
NG0 = 5  # L0 channel groups (4x128 + 16, zero-padded to 5x128)

PE_G = [0, 8, 1, 9]  # L1 groups with PE-built broadcast (one 4-way quad)
DMA_G = [g for g in range(G1) if g not in PE_G]
GPS_MULS = (13, 14, 15)  # z-mul groups routed to GpSimd
# evac engine split for PE-built bc tiles: True -> DVE copy, False -> ACT
BC_EVAC_DVE = (True, False, True, False)

_CACHE = {}


def _build_nc(BS=BS):
    import concourse.bass as bass
    import concourse.tile as tile
    from concourse import bacc, mybir

    T = BS * D
    NPAIR = T // PAIR
    SPP = PAIR // D  # samples per pair

    bf16 = mybir.dt.bfloat16
    f32 = mybir.dt.float32
    Relu = mybir.ActivationFunctionType.Relu
    Square = mybir.ActivationFunctionType.Square
    X = mybir.AxisListType.X
    ADD = mybir.AluOpType.add

    nc = bacc.Bacc(None, target_bir_lowering=False)

    xt = nc.dram_tensor("xt", [128, T], bf16, kind="ExternalInput")
    a0 = nc.dram_tensor("a0", [128, NG0, 128], bf16, kind="ExternalInput")
    w0f = nc.dram_tensor("w0f", [128, NG0, O], bf16, kind="ExternalInput")
    sel1 = nc.dram_tensor("sel1", [128, 1, 128], bf16, kind="ExternalInput")
    w1t = nc.dram_tensor("w1t", [G1 * 128, O], bf16, kind="ExternalInput")
    b0 = nc.dram_tensor("b0", [O, 1], f32, kind="ExternalInput")
    b1 = nc.dram_tensor("b1", [O, 1], f32, kind="ExternalInput")
    out0 = nc.dram_tensor("out0", [O - H1, BS], f32, kind="ExternalOutput")
    out1 = nc.dram_tensor("out1", [O, BS], f32, kind="ExternalOutput")

    with tile.TileContext(nc) as tc:
        with (
            tc.tile_pool(name="singles", bufs=1) as singles,
            tc.tile_pool(name="xrp", bufs=3) as xrp,
            tc.tile_pool(name="s0p", bufs=6) as s0p,
            tc.tile_pool(name="hxq", bufs=2) as hxqp,
            tc.tile_pool(name="hxd", bufs=7) as hxdp,
            tc.tile_pool(name="zp", bufs=17) as zp,
            tc.tile_pool(name="y0sbp", bufs=2) as y0sbp,
            tc.tile_pool(name="y1sbp", bufs=2) as y1sbp,
            tc.tile_pool(name="hdupp", bufs=2) as hdupp,
            tc.tile_pool(name="bcps", bufs=3, space="PSUM") as bcps,
            tc.tile_pool(name="yps", bufs=2, space="PSUM") as yps,
            tc.tile_pool(name="dstp", bufs=2, space="DRAM") as dstp,
        ):
            a0s = singles.tile([128, NG0, 128], bf16)
            w0fs = singles.tile([128, NG0, O], bf16)
            s1s = singles.tile([128, 1, 128], bf16)
            w1s = singles.tile([128, G1, O], bf16)
            b0s = singles.tile([O, 1], f32)
            b1s = singles.tile([O, 1], f32)
            oacc0 = singles.tile([O, BS], f32)
            oacc1 = singles.tile([O, BS], f32)

            nc.gpsimd.dma_start(out=a0s[:], in_=a0[:])
            nc.gpsimd.dma_start(out=w0fs[:], in_=w0f[:])
            nc.gpsimd.dma_start(out=s1s[:], in_=sel1[:])
            nc.gpsimd.dma_start(out=w1s[:], in_=w1t.rearrange("(g k) m -> k g m", k=128))
            nc.gpsimd.dma_start(out=b0s[:], in_=b0[:])
            nc.gpsimd.dma_start(out=b1s[:], in_=b1[:])

            for P in range(NPAIR):
                sl = slice(P * PAIR, (P + 1) * PAIR)
                xr = xrp.tile([128, PAIR], bf16, name=f"xr{P}", tag="xr")
                nc.gpsimd.dma_start(out=xr[:], in_=xt[:, sl])

                # ---- L0: build 528 sum/square channels, K=32 4-way ----
                s0g = []
                for G in range(NG0):
                    psA = bcps.tile([128, 2, 512], f32, name=f"l0psA{P}_{G}", tag="bc")
                    psB = bcps.tile([128, 2, 512], f32, name=f"l0psB{P}_{G}", tag="bc")
                    for c in range(4):
                        ps = psA if c < 2 else psB
                        nc.tensor.matmul(
                            ps[:, c % 2, :],
                            a0s[32 * c : 32 * c + 32, G, :],
                            xr[32 * c : 32 * c + 32, 512 * c : 512 * (c + 1)],
                            start=True, stop=True, tile_position=(32 * c, 0),
                        )
                    sg = s0p.tile([128, PAIR], bf16, name=f"s0_{P}_{G}", tag="s0")
                    for half, ps in ((0, psA), (1, psB)):
                        dst = sg[:, 1024 * half : 1024 * (half + 1)].rearrange(
                            "p (j c) -> p j c", j=2
                        )
                        nc.scalar.activation(dst, ps[:], Square)
                    s0g.append(sg)

                # ---- L0 contract: y0 = w0f.T @ s + b0, relu ----
                y0sb = y0sbp.tile([128, PAIR], bf16, name=f"y0sb{P}", tag="y0sb")
                for q in range(4):
                    qs = slice(512 * q, 512 * (q + 1))
                    y0p = yps.tile([128, 512], f32, name=f"y0p{P}_{q}", tag="yp")
                    for G in range(NG0):
                        nc.tensor.matmul(
                            y0p[:], w0fs[:, G, :], s0g[G][:, qs],
                            start=(G == 0), stop=(G == NG0 - 1),
                        )
                    nc.scalar.activation(y0sb[:, qs], y0p[:], Relu, bias=b0s[:])

                nc.vector.tensor_reduce(
                    oacc0[H1:O, P * SPP : (P + 1) * SPP],
                    y0sb[H1:O, :].rearrange("p (b d) -> p b d", d=D),
                    axis=X, op=ADD,
                )

                # hidden rows to HBM (for DMA broadcast-expansion) + dup to 64:128
                y0st = dstp.tile([H1, PAIR], bf16, space="DRAM", name=f"y0st{P}", tag="y0st")
                nc.gpsimd.dma_start(out=y0st[:], in_=y0sb[0:H1, :])
                hdup = hdupp.tile([128, PAIR], bf16, name=f"hdup{P}", tag="hdup")
                nc.gpsimd.dma_start(out=hdup[64:128, :], in_=y0sb[0:H1, :])

                # ---- L1 broadcast tiles ----
                hxs = {}
                # PE-built quad: groups (0, 8) -> psA strips 0/1, (1, 9) -> psB strips 2/3
                hxq0 = hxqp.tile([128, 2, PAIR], bf16, name=f"hxq0_{P}", tag="hxq")
                hxq1 = hxqp.tile([128, 2, PAIR], bf16, name=f"hxq1_{P}", tag="hxq")
                for c in range(4):
                    cs = slice(512 * c, 512 * (c + 1))
                    psA = bcps.tile([128, 2, 512], f32, name=f"bcpsA{P}_{c}", tag="bc")
                    psB = bcps.tile([128, 2, 512], f32, name=f"bcpsB{P}_{c}", tag="bc")
                    nc.tensor.matmul(
                        psA[:, 0, :], s1s[0:32, 0, :], y0sb[0:32, cs],
                        start=True, stop=True, tile_position=(0, 0),
                    )
                    nc.tensor.matmul(
                        psA[:, 1, :], s1s[32:64, 0, :], y0sb[32:64, cs],
                        start=True, stop=True, tile_position=(32, 0),
                    )
                    nc.tensor.matmul(
                        psB[:, 0, :], s1s[64:96, 0, :], hdup[64:96, cs],
                        start=True, stop=True, tile_position=(64, 0),
                    )
                    nc.tensor.matmul(
                        psB[:, 1, :], s1s[96:128, 0, :], hdup[96:128, cs],
                        start=True, stop=True, tile_position=(96, 0),
                    )
                    for k, (ps, hxq) in enumerate(((psA, hxq0), (psB, hxq1))):
                        dst = hxq[:, :, cs]
                        if BC_EVAC_DVE[2 * c % 4 + k]:
                            nc.vector.tensor_copy(dst, ps[:])
                        else:
                            nc.scalar.activation(
                                dst, ps[:], mybir.ActivationFunctionType.Copy
                            )
                hxs[PE_G[0]] = hxq0[:, 0, :]
                hxs[PE_G[1]] = hxq0[:, 1, :]
                hxs[PE_G[2]] = hxq1[:, 0, :]
                hxs[PE_G[3]] = hxq1[:, 1, :]

                # DMA-expanded groups: read hidden rows 4g..4g+3, replicated 32x
                y0b = y0st.rearrange("(a h) t -> a h t", a=1).broadcast_to(
                    (32, H1, PAIR)
                )
                for g in DMA_G:
                    dhx = hxdp.tile([128, PAIR], bf16, name=f"hxd{P}_{g}", tag="hxd")
                    src = y0b[:, 4 * g : 4 * g + 4, :].rearrange("r h t -> h r t")
                    nc.sync.dma_start(out=dhx[:], in_=src)
                    hxs[g] = dhx[:]

                # ---- z muls ----
                zs = []
                for g in range(G1):
                    z = zp.tile([128, PAIR], bf16, name=f"z{P}_{g}", tag="z")
                    eng = nc.gpsimd if g in GPS_MULS else nc.vector
                    eng.tensor_mul(z[:], xr[:], hxs[g])
                    zs.append(z)

                # ---- L1 contract: y1 = w1.T @ z + b1, relu ----
                y1sb = y1sbp.tile([128, PAIR], bf16, name=f"y1sb{P}", tag="y1sb")
                for q in range(4):
                    qs = slice(512 * q, 512 * (q + 1))
                    y1p = yps.tile([128, 512], f32, name=f"y1p{P}_{q}", tag="yp")
                    for g in range(G1):
                        nc.tensor.matmul(
                            y1p[:], w1s[:, g, :], zs[g][:, qs],
                            start=(g == 0), stop=(g == G1 - 1),
                        )
                    nc.scalar.activation(y1sb[:, qs], y1p[:], Relu, bias=b1s[:])

                nc.vector.tensor_reduce(
                    oacc1[:, P * SPP : (P + 1) * SPP],
                    y1sb[:].rearrange("p (b d) -> p b d", d=D),
                    axis=X, op=ADD,
                )

            nc.gpsimd.dma_start(out=out0[:], in_=oacc0[H1:O, :])
            nc.gpsimd.dma_start(out=out1[:], in_=oacc1[:])

    nc.finalize()
    return nc


def _get_nc():
    if "nc" not in _CACHE:
        _CACHE["nc"] = _build_nc()
    return _CACHE["nc"]


def _l0_pairs():
    return [(h, f) for h in range(F) for f in range(h + 1, F)]


def make_l0(w0_np):
    """A0 build matrix [32, 640] and folded weights [640, 128] (zero-padded)."""
    pairs = _l0_pairs()
    A0 = np.zeros((F, NG0 * 128), np.float32)
    w0fold = np.zeros((NG0 * 128, O), np.float32)
    for k, (h, f) in enumerate(pairs):
        A0[h, k] = 1.0
        A0[f, k] = 1.0
        w0fold[k] = (w0_np[:, h * F + f] + w0_np[:, f * F + h]) / 2
    for h in range(F):
        k = 496 + h
        A0[h, k] = 1.0
        c = w0_np[:, h * F + h].copy()
        for f in range(F):
            if f != h:
                c -= 0.5 * (w0_np[:, h * F + f] + w0_np[:, f * F + h])
        w0fold[k] = c
    return A0, w0fold


def make_sel1():
    """One-hot selectors for the PE-built quad (groups 0, 8, 1, 9).

    Strip s (partitions 32s..32s+31) selects hidden row 4g + m//32 for
    g = PE_G[s]; strips 0/1 read y0sb (partition == hidden row), strips
    2/3 read hdup (partition == 64 + hidden row).
    """
    sel = np.zeros((128, 1, 128), np.float32)
    for s, g in enumerate(PE_G):
        off = 0 if s < 2 else 64
        for m in range(128):
            sel[off + 4 * g + m // 32, 0, m] = 1.0
    return sel


def kernel(cin_inputs, w0, b0, w1, b1, _trace=False):
    from concourse.bass_utils import run_bass_kernel_spmd

    x = np.asarray(cin_inputs, dtype=np.float32)
    assert x.shape == (B_FULL, F, D)
    bf = ml_dtypes.bfloat16
    # [B, F, D] -> per-core [F, BS*D] bf16, tiled 4x along partitions
    xt_all = np.ascontiguousarray(
        x.reshape(N_CORES, BS, F, D).transpose(0, 2, 1, 3)
    ).astype(bf).reshape(N_CORES, F, BS * D)
    xt_all = np.ascontiguousarray(np.tile(xt_all, (1, 4, 1)))

    w0_np = np.asarray(w0, dtype=np.float32)
    A0, w0fold = make_l0(w0_np)
    # a0: [128, 5, 128] (same 32-row block on each strip)
    a0c = np.ascontiguousarray(
        np.tile(A0.reshape(F, NG0, 128), (4, 1, 1))
    ).astype(bf)
    # w0f: [128, 5, 128]: [k_local, G, o]
    w0fc = np.ascontiguousarray(
        w0fold.reshape(NG0, 128, O).transpose(1, 0, 2)
    ).astype(bf)
    s1 = make_sel1().astype(bf)
    w1t = np.ascontiguousarray(np.asarray(w1, dtype=np.float32).T).astype(bf)
    b0c = np.asarray(b0, dtype=np.float32).reshape(O, 1).copy()
    b1c = np.asarray(b1, dtype=np.float32).reshape(O, 1).copy()

    nc = _get_nc()
    in_maps = []
    for i in range(N_CORES):
        in_maps.append(
            {
                "xt": xt_all[i],
                "a0": a0c, "w0f": w0fc, "sel1": s1, "w1t": w1t,
                "b0": b0c, "b1": b1c,
            }
        )
    res = run_bass_kernel_spmd(nc, in_maps, core_ids=list(range(N_CORES)), trace=_trace)
    outs = []
    for r in res.results:
        o = np.concatenate([r["out0"], r["out1"]], axis=0).T
        outs.append(o)
    full = np.concatenate(outs, axis=0).astype(np.float32)
    if _trace:
        return full, res
    return full


# revision 7
# speedup vs baseline: 1.1049x; 1.1049x over previous
"""
CIN (Compressed Interaction Network) kernel for Trainium2, 8 NeuronCores.

Problem (hardcoded):
  x: [4096, 32, 64] fp32; w0: [128, 1024]; b0: [128]; w1: [128, 2048]; b1: [128]
  out: [4096, 192] = concat(relu(y0)[:, 64:], relu(y1)).sum(d)
  y0 = w0 @ vec(x (x) x) per (b, d) token; y1 = w1 @ vec(hidden (x) x).

Sharding: data parallel over batch, 512 samples/core, tokens t=(b,d), T=32768.

Per-core pipeline (pair = 2048 tokens):
  - L0 uses the polarization identity x_h*x_f = ((x_h+x_f)^2 - x_h^2 - x_f^2)/2:
    496 upper-triangle sum-channels + 32 square-channels = 528 channels
    (vs 1024 raw). u = A0 @ x built on the PE as K=32 two-hot matmuls with
    4-way tile_position concurrency (x tiled 4x across partitions); ScalarE
    evacuates with func=Square; contract with host-folded weights.
  - L1 direct z = hx * xr: hidden rows round-trip through HBM (y0st) and all
    16 broadcast groups are DMA-expanded back with replicating access
    patterns, 2 groups per SWDGE dma (1 MB each, issued from GpSimd so the
    queue cost is ~0.6us and transfers drain asynchronously across the 16
    SDMA engines). z-muls run in place on the expanded tiles (VectorE,
    last tile on GpSimd).
  - y evac ScalarE Relu+bias from [128,512] PSUM quarters; d-sum via a
    log2 tree of strided VectorE adds in place on y?sb, final add lands
    f32 in the output accumulator.
"""

import sys

import numpy as np
import ml_dtypes

sys.path.insert(0, "/opt/trn_rl_repo")

B_FULL = 4096
N_CORES = 8
BS = B_FULL // N_CORES  # 512
F = 32
D = 64
T = BS * D  # 32768
PAIR = 2048  # tokens per pair (32 samples)
O = 128
H1 = 64
G1 = 16
CH0 = 528  # 496 pairs (h<f) + 32 squares
NG0 = 5  # L0 channel groups (4x128 + 16, zero-padded to 5x128)

GPS_MULS = (14, 15)  # z-mul groups routed to GpSimd (one hx tile)

_CACHE = {}


def _build_nc(BS=BS):
    import concourse.bass as bass
    import concourse.tile as tile
    from concourse import bacc, mybir

    T = BS * D
    NPAIR = T // PAIR
    SPP = PAIR // D  # samples per pair

    bf16 = mybir.dt.bfloat16
    f32 = mybir.dt.float32
    Relu = mybir.ActivationFunctionType.Relu
    Square = mybir.ActivationFunctionType.Square

    nc = bacc.Bacc(None, target_bir_lowering=False)

    xt = nc.dram_tensor("xt", [128, T], bf16, kind="ExternalInput")
    a0 = nc.dram_tensor("a0", [128, NG0, 128], bf16, kind="ExternalInput")
    w0f = nc.dram_tensor("w0f", [128, NG0, O], bf16, kind="ExternalInput")
    w1t = nc.dram_tensor("w1t", [G1 * 128, O], bf16, kind="ExternalInput")
    b0 = nc.dram_tensor("b0", [O, 1], f32, kind="ExternalInput")
    b1 = nc.dram_tensor("b1", [O, 1], f32, kind="ExternalInput")
    out0 = nc.dram_tensor("out0", [O - H1, BS], f32, kind="ExternalOutput")
    out1 = nc.dram_tensor("out1", [O, BS], f32, kind="ExternalOutput")

    with tile.TileContext(nc) as tc:
        with (
            tc.tile_pool(name="singles", bufs=1) as singles,
            tc.tile_pool(name="xrp", bufs=3) as xrp,
            tc.tile_pool(name="s0p", bufs=6) as s0p,
            tc.tile_pool(name="hxp", bufs=10) as hxp,
            tc.tile_pool(name="y0sbp", bufs=2) as y0sbp,
            tc.tile_pool(name="y1sbp", bufs=2) as y1sbp,
            tc.tile_pool(name="bcps", bufs=3, space="PSUM") as bcps,
            tc.tile_pool(name="yps", bufs=2, space="PSUM") as yps,
            tc.tile_pool(name="dstp", bufs=2, space="DRAM") as dstp,
        ):
            a0s = singles.tile([128, NG0, 128], bf16)
            w0fs = singles.tile([128, NG0, O], bf16)
            w1s = singles.tile([128, G1, O], bf16)
            b0s = singles.tile([O, 1], f32)
            b1s = singles.tile([O, 1], f32)
            oacc0 = singles.tile([O, BS], f32)
            oacc1 = singles.tile([O, BS], f32)

            nc.gpsimd.dma_start(out=a0s[:], in_=a0[:])
            nc.gpsimd.dma_start(out=w0fs[:], in_=w0f[:])
            nc.gpsimd.dma_start(out=w1s[:], in_=w1t.rearrange("(g k) m -> k g m", k=128))
            nc.gpsimd.dma_start(out=b0s[:], in_=b0[:])
            nc.gpsimd.dma_start(out=b1s[:], in_=b1[:])

            for P in range(NPAIR):
                sl = slice(P * PAIR, (P + 1) * PAIR)
                xr = xrp.tile([128, PAIR], bf16, name=f"xr{P}", tag="xr")
                nc.gpsimd.dma_start(out=xr[:], in_=xt[:, sl])

                # ---- L0: build 528 sum/square channels, K=32 4-way ----
                s0g = []
                for G in range(NG0):
                    psA = bcps.tile([128, 2, 512], f32, name=f"l0psA{P}_{G}", tag="bc")
                    psB = bcps.tile([128, 2, 512], f32, name=f"l0psB{P}_{G}", tag="bc")
                    for c in range(4):
                        ps = psA if c < 2 else psB
                        nc.tensor.matmul(
                            ps[:, c % 2, :],
                            a0s[32 * c : 32 * c + 32, G, :],
                            xr[32 * c : 32 * c + 32, 512 * c : 512 * (c + 1)],
                            start=True, stop=True, tile_position=(32 * c, 0),
                        )
                    sg = s0p.tile([128, PAIR], bf16, name=f"s0_{P}_{G}", tag="s0")
                    for half, ps in ((0, psA), (1, psB)):
                        dst = sg[:, 1024 * half : 1024 * (half + 1)].rearrange(
                            "p (j c) -> p j c", j=2
                        )
                        nc.scalar.activation(dst, ps[:], Square)
                    s0g.append(sg)

                # ---- L0 contract: y0 = w0f.T @ s + b0, relu ----
                y0sb = y0sbp.tile([128, PAIR], bf16, name=f"y0sb{P}", tag="y0sb")
                for q in range(4):
                    qs = slice(512 * q, 512 * (q + 1))
                    y0p = yps.tile([128, 512], f32, name=f"y0p{P}_{q}", tag="yp")
                    for G in range(NG0):
                        nc.tensor.matmul(
                            y0p[:], w0fs[:, G, :], s0g[G][:, qs],
                            start=(G == 0), stop=(G == NG0 - 1),
                        )
                    nc.scalar.activation(y0sb[:, qs], y0p[:], Relu, bias=b0s[:])

                # hidden rows to HBM for DMA broadcast-expansion
                y0st = dstp.tile([H1, PAIR], bf16, space="DRAM", name=f"y0st{P}", tag="y0st")
                nc.gpsimd.dma_start(out=y0st[:], in_=y0sb[0:H1, :])

                # out0 = sum_d relu(y0)[64:128]: log-tree, in place on y0sb
                w = D // 2
                while w >= 1:
                    a = y0sb[H1:O, :].rearrange("p (b d) -> p b d", d=D)
                    dst = (
                        oacc0[H1:O, P * SPP : (P + 1) * SPP]
                        if w == 1
                        else a[:, :, 0:w]
                    )
                    nc.vector.tensor_add(dst, a[:, :, 0:w], a[:, :, w : 2 * w])
                    w //= 2

                # ---- L1: DMA-expand all 16 broadcast groups (2 per dma) ----
                y0b = y0st.rearrange("(a h) t -> a h t", a=1).broadcast_to(
                    (32, H1, PAIR)
                )
                hxs = []
                for R in range(8):
                    hx = hxp.tile([128, 2, PAIR], bf16, name=f"hx{P}_{R}", tag="hx")
                    hxs.append(hx)
                for g in range(G1):
                    src = y0b[:, 4 * g : 4 * g + 4, :].rearrange("r h t -> h r t")
                    nc.gpsimd.dma_start(out=hxs[g // 2][:, g % 2, :], in_=src)

                # ---- z muls, in place: hx[:, j, :] *= xr ----
                for g in range(G1):
                    hxg = hxs[g // 2][:, g % 2, :]
                    eng = nc.gpsimd if g in GPS_MULS else nc.vector
                    eng.tensor_mul(hxg, hxg, xr[:])

                # ---- L1 contract: y1 = w1.T @ z + b1, relu ----
                y1sb = y1sbp.tile([128, PAIR], bf16, name=f"y1sb{P}", tag="y1sb")
                for q in range(4):
                    qs = slice(512 * q, 512 * (q + 1))
                    y1p = yps.tile([128, 512], f32, name=f"y1p{P}_{q}", tag="yp")
                    for g in range(G1):
                        nc.tensor.matmul(
                            y1p[:], w1s[:, g, :], hxs[g // 2][:, g % 2, qs],
                            start=(g == 0), stop=(g == G1 - 1),
                        )
                    nc.scalar.activation(y1sb[:, qs], y1p[:], Relu, bias=b1s[:])

                # out1 = sum_d relu(y1): log-tree, in place on y1sb
                w = D // 2
                while w >= 1:
                    a = y1sb[:].rearrange("p (b d) -> p b d", d=D)
                    dst = (
                        oacc1[:, P * SPP : (P + 1) * SPP] if w == 1 else a[:, :, 0:w]
                    )
                    nc.vector.tensor_add(dst, a[:, :, 0:w], a[:, :, w : 2 * w])
                    w //= 2

            nc.gpsimd.dma_start(out=out0[:], in_=oacc0[H1:O, :])
            nc.gpsimd.dma_start(out=out1[:], in_=oacc1[:])

    nc.finalize()
    return nc


def _get_nc():
    if "nc" not in _CACHE:
        _CACHE["nc"] = _build_nc()
    return _CACHE["nc"]


def _l0_pairs():
    return [(h, f) for h in range(F) for f in range(h + 1, F)]


def make_l0(w0_np):
    """A0 build matrix [32, 640] and folded weights [640, 128] (zero-padded)."""
    pairs = _l0_pairs()
    A0 = np.zeros((F, NG0 * 128), np.float32)
    w0fold = np.zeros((NG0 * 128, O), np.float32)
    for k, (h, f) in enumerate(pairs):
        A0[h, k] = 1.0
        A0[f, k] = 1.0
        w0fold[k] = (w0_np[:, h * F + f] + w0_np[:, f * F + h]) / 2
    for h in range(F):
        k = 496 + h
        A0[h, k] = 1.0
        c = w0_np[:, h * F + h].copy()
        for f in range(F):
            if f != h:
                c -= 0.5 * (w0_np[:, h * F + f] + w0_np[:, f * F + h])
        w0fold[k] = c
    return A0, w0fold


def kernel(cin_inputs, w0, b0, w1, b1, _trace=False):
    from concourse.bass_utils import run_bass_kernel_spmd

    x = np.asarray(cin_inputs, dtype=np.float32)
    assert x.shape == (B_FULL, F, D)
    bf = ml_dtypes.bfloat16
    # [B, F, D] -> per-core [F, BS*D] bf16, tiled 4x along partitions
    xt_all = np.ascontiguousarray(
        x.reshape(N_CORES, BS, F, D).transpose(0, 2, 1, 3)
    ).astype(bf).reshape(N_CORES, F, BS * D)
    xt_all = np.ascontiguousarray(np.tile(xt_all, (1, 4, 1)))

    w0_np = np.asarray(w0, dtype=np.float32)
    A0, w0fold = make_l0(w0_np)
    a0c = np.ascontiguousarray(
        np.tile(A0.reshape(F, NG0, 128), (4, 1, 1))
    ).astype(bf)
    w0fc = np.ascontiguousarray(
        w0fold.reshape(NG0, 128, O).transpose(1, 0, 2)
    ).astype(bf)
    w1tc = np.ascontiguousarray(np.asarray(w1, dtype=np.float32).T).astype(bf)
    b0c = np.asarray(b0, dtype=np.float32).reshape(O, 1).copy()
    b1c = np.asarray(b1, dtype=np.float32).reshape(O, 1).copy()

    nc = _get_nc()
    in_maps = []
    for i in range(N_CORES):
        in_maps.append(
            {
                "xt": xt_all[i],
                "a0": a0c, "w0f": w0fc, "w1t": w1tc,
                "b0": b0c, "b1": b1c,
            }
        )
    res = run_bass_kernel_spmd(nc, in_maps, core_ids=list(range(N_CORES)), trace=_trace)
    outs = []
    for r in res.results:
        o = np.concatenate([r["out0"], r["out1"]], axis=0).T
        outs.append(o)
    full = np.concatenate(outs, axis=0).astype(np.float32)
    if _trace:
        return full, res
    return full


# revision 8
# speedup vs baseline: 1.3453x; 1.2176x over previous
"""
CIN (Compressed Interaction Network) kernel for Trainium2, 8 NeuronCores.

Problem (hardcoded):
  x: [4096, 32, 64] fp32; w0: [128, 1024]; b0: [128]; w1: [128, 2048]; b1: [128]
  out: [4096, 192] = concat(relu(y0)[:, 64:], relu(y1)).sum(d)
  y0 = w0 @ vec(x (x) x) per (b, d) token; y1 = w1 @ vec(hidden (x) x).

Sharding: data parallel over batch, 512 samples/core, tokens t=(b,d), T=32768.

Per-core pipeline (pair = 2048 tokens), software-pipelined so block P runs
L0 of pair P and L1 of pair P-1:
  - L0 via the polarization identity x_h*x_f = ((x_h+x_f)^2 - x_h^2 - x_f^2)/2:
    496 upper-triangle sum-channels + 32 square-channels = 528 (vs 1024 raw).
    u = A0 @ x built on the PE as K=32 two-hot matmuls, 4-way tile_position
    concurrent; ScalarE evacuates with func=Square; host-folded weights.
  - L1 direct z = hx * xr: hidden rows round-trip through HBM (y0st) and all
    16 broadcast groups are DMA-expanded with replicating access patterns
    (SWDGE from GpSimd, async). Expansions for pair P are issued mid-block P;
    the muls consume them one block later, so transfers fully hide.
  - z-muls run in place on the expanded tiles (VectorE x14, GpSimd x2);
    contracts are group-major into 4 resident PSUM quarter tiles; y evac
    ScalarE Relu+bias; d-sum via a log2 tree of strided VectorE adds.
"""

import sys

import numpy as np
import ml_dtypes

sys.path.insert(0, "/opt/trn_rl_repo")

B_FULL = 4096
N_CORES = 8
BS = B_FULL // N_CORES  # 512
F = 32
D = 64
T = BS * D  # 32768
PAIR = 2048  # tokens per pair (32 samples)
O = 128
H1 = 64
G1 = 16
CH0 = 528
NG0 = 5  # L0 channel groups (4x128 + 16, zero-padded to 5x128)

GPS_MULS = (14, 15)  # z-mul groups routed to GpSimd

_CACHE = {}


def _build_nc(BS=BS):
    import concourse.bass as bass
    import concourse.tile as tile
    from concourse import bacc, mybir

    T = BS * D
    NPAIR = T // PAIR
    SPP = PAIR // D

    bf16 = mybir.dt.bfloat16
    f32 = mybir.dt.float32
    Relu = mybir.ActivationFunctionType.Relu
    Square = mybir.ActivationFunctionType.Square

    nc = bacc.Bacc(None, target_bir_lowering=False)

    xt = nc.dram_tensor("xt", [128, T], bf16, kind="ExternalInput")
    a0 = nc.dram_tensor("a0", [128, NG0, 128], bf16, kind="ExternalInput")
    w0f = nc.dram_tensor("w0f", [128, NG0, O], bf16, kind="ExternalInput")
    w1t = nc.dram_tensor("w1t", [G1 * 128, O], bf16, kind="ExternalInput")
    b0 = nc.dram_tensor("b0", [O, 1], f32, kind="ExternalInput")
    b1 = nc.dram_tensor("b1", [O, 1], f32, kind="ExternalInput")
    out0 = nc.dram_tensor("out0", [O - H1, BS], f32, kind="ExternalOutput")
    out1 = nc.dram_tensor("out1", [O, BS], f32, kind="ExternalOutput")

    with tile.TileContext(nc) as tc:
        with (
            tc.tile_pool(name="singles", bufs=1) as singles,
            tc.tile_pool(name="xrp", bufs=3) as xrp,
            tc.tile_pool(name="s0p", bufs=6) as s0p,
            tc.tile_pool(name="hxp", bufs=10) as hxp,
            tc.tile_pool(name="y0sbp", bufs=2) as y0sbp,
            tc.tile_pool(name="y1sbp", bufs=2) as y1sbp,
            tc.tile_pool(name="bcps", bufs=2, space="PSUM") as bcps,
            tc.tile_pool(name="yqp", bufs=4, space="PSUM") as yqp,
            tc.tile_pool(name="dstp", bufs=2, space="DRAM") as dstp,
        ):
            a0s = singles.tile([128, NG0, 128], bf16)
            w0fs = singles.tile([128, NG0, O], bf16)
            w1s = singles.tile([128, G1, O], bf16)
            b0s = singles.tile([O, 1], f32)
            b1s = singles.tile([O, 1], f32)
            oacc0 = singles.tile([O, BS], f32)
            oacc1 = singles.tile([O, BS], f32)

            nc.gpsimd.dma_start(out=a0s[:], in_=a0[:])
            nc.gpsimd.dma_start(out=w0fs[:], in_=w0f[:])
            nc.gpsimd.dma_start(out=w1s[:], in_=w1t.rearrange("(g k) m -> k g m", k=128))
            nc.gpsimd.dma_start(out=b0s[:], in_=b0[:])
            nc.gpsimd.dma_start(out=b1s[:], in_=b1[:])

            def do_l1(P, xr, hxs):
                """muls + L1 contract + y1 evac + out1 tree for pair P."""
                for g in range(G1):
                    hxg = hxs[g // 2][:, g % 2, :]
                    eng = nc.gpsimd if g in GPS_MULS else nc.vector
                    eng.tensor_mul(hxg, hxg, xr[:])
                y1sb = y1sbp.tile([128, PAIR], bf16, name=f"y1sb{P}", tag="y1sb")
                yq = [
                    yqp.tile([128, 512], f32, name=f"y1q{P}_{q}", tag="yq")
                    for q in range(4)
                ]
                for g in range(G1):
                    for q in range(4):
                        nc.tensor.matmul(
                            yq[q][:], w1s[:, g, :],
                            hxs[g // 2][:, g % 2, 512 * q : 512 * (q + 1)],
                            start=(g == 0), stop=(g == G1 - 1),
                        )
                for q in range(4):
                    nc.scalar.activation(
                        y1sb[:, 512 * q : 512 * (q + 1)], yq[q][:], Relu, bias=b1s[:]
                    )
                w = D // 2
                while w >= 1:
                    a = y1sb[:].rearrange("p (b d) -> p b d", d=D)
                    dst = (
                        oacc1[:, P * SPP : (P + 1) * SPP] if w == 1 else a[:, :, 0:w]
                    )
                    nc.vector.tensor_add(dst, a[:, :, 0:w], a[:, :, w : 2 * w])
                    w //= 2

            prev = None  # (P-1, xr, hxs)
            for P in range(NPAIR):
                sl = slice(P * PAIR, (P + 1) * PAIR)
                xr = xrp.tile([128, PAIR], bf16, name=f"xr{P}", tag="xr")
                nc.sync.dma_start(out=xr[:], in_=xt[:, sl])

                # ---- L0(P) builds + square evacs ----
                s0g = []
                for G in range(NG0):
                    psA = bcps.tile([128, 2, 512], f32, name=f"l0psA{P}_{G}", tag="bc")
                    psB = bcps.tile([128, 2, 512], f32, name=f"l0psB{P}_{G}", tag="bc")
                    for c in range(4):
                        ps = psA if c < 2 else psB
                        nc.tensor.matmul(
                            ps[:, c % 2, :],
                            a0s[32 * c : 32 * c + 32, G, :],
                            xr[32 * c : 32 * c + 32, 512 * c : 512 * (c + 1)],
                            start=True, stop=True, tile_position=(32 * c, 0),
                        )
                    sg = s0p.tile([128, PAIR], bf16, name=f"s0_{P}_{G}", tag="s0")
                    for half, ps in ((0, psA), (1, psB)):
                        dst = sg[:, 1024 * half : 1024 * (half + 1)].rearrange(
                            "p (j c) -> p j c", j=2
                        )
                        nc.scalar.activation(dst, ps[:], Square)
                    s0g.append(sg)

                # ---- L1(P-1): muls, contract, evac, tree ----
                if prev is not None:
                    do_l1(*prev)

                # ---- L0(P) contract (group-major into 4 quarter tiles) ----
                y0sb = y0sbp.tile([128, PAIR], bf16, name=f"y0sb{P}", tag="y0sb")
                yq = [
                    yqp.tile([128, 512], f32, name=f"y0q{P}_{q}", tag="yq")
                    for q in range(4)
                ]
                for G in range(NG0):
                    for q in range(4):
                        nc.tensor.matmul(
                            yq[q][:], w0fs[:, G, :],
                            s0g[G][:, 512 * q : 512 * (q + 1)],
                            start=(G == 0), stop=(G == NG0 - 1),
                        )
                for q in range(4):
                    nc.scalar.activation(
                        y0sb[:, 512 * q : 512 * (q + 1)], yq[q][:], Relu, bias=b0s[:]
                    )

                # ---- out0 tree + hidden store + expansions(P) ----
                w = D // 2
                while w >= 1:
                    a = y0sb[H1:O, :].rearrange("p (b d) -> p b d", d=D)
                    dst = (
                        oacc0[H1:O, P * SPP : (P + 1) * SPP]
                        if w == 1
                        else a[:, :, 0:w]
                    )
                    nc.vector.tensor_add(dst, a[:, :, 0:w], a[:, :, w : 2 * w])
                    w //= 2

                y0st = dstp.tile([H1, PAIR], bf16, space="DRAM", name=f"y0st{P}", tag="y0st")
                nc.sync.dma_start(out=y0st[:], in_=y0sb[0:H1, :])
                y0b = y0st.rearrange("(a h) t -> a h t", a=1).broadcast_to(
                    (32, H1, PAIR)
                )
                hxs = []
                for R in range(8):
                    hx = hxp.tile([128, 2, PAIR], bf16, name=f"hx{P}_{R}", tag="hx")
                    hxs.append(hx)
                for g in range(G1):
                    src = y0b[:, 4 * g : 4 * g + 4, :].rearrange("r h t -> h r t")
                    nc.gpsimd.dma_start(out=hxs[g // 2][:, g % 2, :], in_=src)

                prev = (P, xr, hxs)

            do_l1(*prev)

            nc.gpsimd.dma_start(out=out0[:], in_=oacc0[H1:O, :])
            nc.gpsimd.dma_start(out=out1[:], in_=oacc1[:])

    nc.finalize()
    return nc


def _get_nc():
    if "nc" not in _CACHE:
        _CACHE["nc"] = _build_nc()
    return _CACHE["nc"]


def _l0_pairs():
    return [(h, f) for h in range(F) for f in range(h + 1, F)]


def make_l0(w0_np):
    """A0 build matrix [32, 640] and folded weights [640, 128] (zero-padded)."""
    pairs = _l0_pairs()
    A0 = np.zeros((F, NG0 * 128), np.float32)
    w0fold = np.zeros((NG0 * 128, O), np.float32)
    for k, (h, f) in enumerate(pairs):
        A0[h, k] = 1.0
        A0[f, k] = 1.0
        w0fold[k] = (w0_np[:, h * F + f] + w0_np[:, f * F + h]) / 2
    for h in range(F):
        k = 496 + h
        A0[h, k] = 1.0
        c = w0_np[:, h * F + h].copy()
        for f in range(F):
            if f != h:
                c -= 0.5 * (w0_np[:, h * F + f] + w0_np[:, f * F + h])
        w0fold[k] = c
    return A0, w0fold


def kernel(cin_inputs, w0, b0, w1, b1, _trace=False):
    from concourse.bass_utils import run_bass_kernel_spmd

    x = np.asarray(cin_inputs, dtype=np.float32)
    assert x.shape == (B_FULL, F, D)
    bf = ml_dtypes.bfloat16
    xt_all = np.ascontiguousarray(
        x.reshape(N_CORES, BS, F, D).transpose(0, 2, 1, 3)
    ).astype(bf).reshape(N_CORES, F, BS * D)
    xt_all = np.ascontiguousarray(np.tile(xt_all, (1, 4, 1)))

    w0_np = np.asarray(w0, dtype=np.float32)
    A0, w0fold = make_l0(w0_np)
    a0c = np.ascontiguousarray(
        np.tile(A0.reshape(F, NG0, 128), (4, 1, 1))
    ).astype(bf)
    w0fc = np.ascontiguousarray(
        w0fold.reshape(NG0, 128, O).transpose(1, 0, 2)
    ).astype(bf)
    w1tc = np.ascontiguousarray(np.asarray(w1, dtype=np.float32).T).astype(bf)
    b0c = np.asarray(b0, dtype=np.float32).reshape(O, 1).copy()
    b1c = np.asarray(b1, dtype=np.float32).reshape(O, 1).copy()

    nc = _get_nc()
    in_maps = []
    for i in range(N_CORES):
        in_maps.append(
            {
                "xt": xt_all[i],
                "a0": a0c, "w0f": w0fc, "w1t": w1tc,
                "b0": b0c, "b1": b1c,
            }
        )
    res = run_bass_kernel_spmd(nc, in_maps, core_ids=list(range(N_CORES)), trace=_trace)
    outs = []
    for r in res.results:
        o = np.concatenate([r["out0"], r["out1"]], axis=0).T
        outs.append(o)
    full = np.concatenate(outs, axis=0).astype(np.float32)
    if _trace:
        return full, res
    return full


# revision 9
# speedup vs baseline: 1.3706x; 1.0188x over previous
"""
CIN (Compressed Interaction Network) kernel for Trainium2, 8 NeuronCores.

Problem (hardcoded):
  x: [4096, 32, 64] fp32; w0: [128, 1024]; b0: [128]; w1: [128, 2048]; b1: [128]
  out: [4096, 192] = concat(relu(y0)[:, 64:], relu(y1)).sum(d)

Sharding: data parallel over batch, 512 samples/core, tokens t=(b,d), T=32768.

Per-core schedule: depth-2 software pipeline over token pairs (2048 tokens).
Block P emits stage S1 of pair P (x load + L0 channel builds), stage S2 of
pair P-1 (L0 contract, hidden store, broadcast expansions) and stage S3 of
pair P-2 (z muls + L1 contract), with the five S1 build groups interleaved
between contract bursts so the in-order PE queue never stalls on the
ScalarE-gated PSUM slots. Every consumer therefore runs a full block (~20us)
after its producer and the PE stays HAM-warm.

  - L0 via polarization: x_h*x_f = ((x_h+x_f)^2 - x_h^2 - x_f^2)/2 ->
    496 upper-triangle sum-channels + 32 squares = 528 channels (vs 1024).
    Built on the PE as K=32 two-hot matmuls, 4-way tile_position concurrent;
    ScalarE evacuates with func=Square; host-folded contract weights.
  - L1 direct z = hx * xr: hidden rows round-trip through HBM (y0st); all 16
    broadcast groups DMA-expanded with replicating access patterns (SWDGE
    from GpSimd, async). Muls run in place on the expanded tiles (VectorE
    x14, GpSimd x2).
  - Contracts are group-major into 4 resident PSUM quarter tiles with one
    explicit LDWEIGHTS per group (matmuls carry ldweights=False).
  - d-sum via log2 tree of strided VectorE adds, final add lands f32.
"""

import sys

import numpy as np
import ml_dtypes

sys.path.insert(0, "/opt/trn_rl_repo")

B_FULL = 4096
N_CORES = 8
BS = B_FULL // N_CORES  # 512
F = 32
D = 64
T = BS * D
PAIR = 2048
O = 128
H1 = 64
G1 = 16
CH0 = 528
NG0 = 5

GPS_MULS = (14, 15)

_CACHE = {}


def _build_nc(BS=BS):
    import concourse.bass as bass
    import concourse.tile as tile
    from concourse import bacc, mybir

    T = BS * D
    NPAIR = T // PAIR
    SPP = PAIR // D

    bf16 = mybir.dt.bfloat16
    f32 = mybir.dt.float32
    Relu = mybir.ActivationFunctionType.Relu
    Square = mybir.ActivationFunctionType.Square

    nc = bacc.Bacc(None, target_bir_lowering=False)

    xt = nc.dram_tensor("xt", [128, T], bf16, kind="ExternalInput")
    a0 = nc.dram_tensor("a0", [128, NG0, 128], bf16, kind="ExternalInput")
    w0f = nc.dram_tensor("w0f", [128, NG0, O], bf16, kind="ExternalInput")
    w1t = nc.dram_tensor("w1t", [G1 * 128, O], bf16, kind="ExternalInput")
    b0 = nc.dram_tensor("b0", [O, 1], f32, kind="ExternalInput")
    b1 = nc.dram_tensor("b1", [O, 1], f32, kind="ExternalInput")
    out0 = nc.dram_tensor("out0", [O - H1, BS], f32, kind="ExternalOutput")
    out1 = nc.dram_tensor("out1", [O, BS], f32, kind="ExternalOutput")

    with tile.TileContext(nc) as tc:
        with (
            tc.tile_pool(name="singles", bufs=1) as singles,
            tc.tile_pool(name="xrp", bufs=4) as xrp,
            tc.tile_pool(name="s0p", bufs=10) as s0p,
            tc.tile_pool(name="hxp", bufs=11) as hxp,
            tc.tile_pool(name="y0sbp", bufs=2) as y0sbp,
            tc.tile_pool(name="y1sbp", bufs=2) as y1sbp,
            tc.tile_pool(name="bcps", bufs=2, space="PSUM") as bcps,
            tc.tile_pool(name="yqp", bufs=4, space="PSUM") as yqp,
            tc.tile_pool(name="dstp", bufs=2, space="DRAM") as dstp,
        ):
            a0s = singles.tile([128, NG0, 128], bf16)
            w0fs = singles.tile([128, NG0, O], bf16)
            w1s = singles.tile([128, G1, O], bf16)
            b0s = singles.tile([O, 1], f32)
            b1s = singles.tile([O, 1], f32)
            oacc0 = singles.tile([O, BS], f32)
            oacc1 = singles.tile([O, BS], f32)

            nc.gpsimd.dma_start(out=a0s[:], in_=a0[:])
            nc.gpsimd.dma_start(out=w0fs[:], in_=w0f[:])
            nc.gpsimd.dma_start(out=w1s[:], in_=w1t.rearrange("(g k) m -> k g m", k=128))
            nc.gpsimd.dma_start(out=b0s[:], in_=b0[:])
            nc.gpsimd.dma_start(out=b1s[:], in_=b1[:])

            st = {}  # per-pair state

            def mm_nold(out_ap, lhsT, rhs, **kw):
                m = nc.tensor.matmul(out_ap, lhsT, rhs, **kw)
                m.ins.ldweights = False
                return m

            def s1_load(P):
                xr = xrp.tile([128, PAIR], bf16, name=f"xr{P}", tag="xr")
                nc.sync.dma_start(out=xr[:], in_=xt[:, P * PAIR : (P + 1) * PAIR])
                st[P] = {"xr": xr, "s0": []}

            def s1_build(P, G):
                xr = st[P]["xr"]
                psA = bcps.tile([128, 2, 512], f32, name=f"psA{P}_{G}", tag="bc")
                psB = bcps.tile([128, 2, 512], f32, name=f"psB{P}_{G}", tag="bc")
                for c in range(4):
                    ps = psA if c < 2 else psB
                    nc.tensor.matmul(
                        ps[:, c % 2, :],
                        a0s[32 * c : 32 * c + 32, G, :],
                        xr[32 * c : 32 * c + 32, 512 * c : 512 * (c + 1)],
                        start=True, stop=True, tile_position=(32 * c, 0),
                    )
                sg = s0p.tile([128, PAIR], bf16, name=f"s0_{P}_{G}", tag="s0")
                for half, ps in ((0, psA), (1, psB)):
                    dst = sg[:, 1024 * half : 1024 * (half + 1)].rearrange(
                        "p (j c) -> p j c", j=2
                    )
                    nc.scalar.activation(dst, ps[:], Square)
                st[P]["s0"].append(sg)

            def s2(P):
                """L0 contract + y0 evac + out0 tree + hidden store + expansions."""
                s0g = st[P]["s0"]
                y0sb = y0sbp.tile([128, PAIR], bf16, name=f"y0sb{P}", tag="y0sb")
                yq = [
                    yqp.tile([128, 512], f32, name=f"y0q{P}_{q}", tag="yq")
                    for q in range(4)
                ]
                for G in range(NG0):
                    nc.tensor.ldweights(w0fs[:, G, :])
                    for q in range(4):
                        mm_nold(
                            yq[q][:], w0fs[:, G, :],
                            s0g[G][:, 512 * q : 512 * (q + 1)],
                            start=(G == 0), stop=(G == NG0 - 1),
                        )
                for q in range(4):
                    nc.scalar.activation(
                        y0sb[:, 512 * q : 512 * (q + 1)], yq[q][:], Relu, bias=b0s[:]
                    )
                w = D // 2
                while w >= 1:
                    a = y0sb[H1:O, :].rearrange("p (b d) -> p b d", d=D)
                    dst = (
                        oacc0[H1:O, P * SPP : (P + 1) * SPP]
                        if w == 1
                        else a[:, :, 0:w]
                    )
                    nc.vector.tensor_add(dst, a[:, :, 0:w], a[:, :, w : 2 * w])
                    w //= 2
                y0st = dstp.tile(
                    [H1, PAIR], bf16, space="DRAM", name=f"y0st{P}", tag="y0st"
                )
                nc.sync.dma_start(out=y0st[:], in_=y0sb[0:H1, :])
                y0b = y0st.rearrange("(a h) t -> a h t", a=1).broadcast_to(
                    (32, H1, PAIR)
                )
                hxs = []
                for R in range(8):
                    hx = hxp.tile([128, 2, PAIR], bf16, name=f"hx{P}_{R}", tag="hx")
                    hxs.append(hx)
                for g in range(G1):
                    src = y0b[:, 4 * g : 4 * g + 4, :].rearrange("r h t -> h r t")
                    nc.gpsimd.dma_start(out=hxs[g // 2][:, g % 2, :], in_=src)
                st[P]["hxs"] = hxs

            def s3_muls(P):
                xr, hxs = st[P]["xr"], st[P]["hxs"]
                for g in range(G1):
                    hxg = hxs[g // 2][:, g % 2, :]
                    eng = nc.gpsimd if g in GPS_MULS else nc.vector
                    eng.tensor_mul(hxg, hxg, xr[:])
                y1sb = y1sbp.tile([128, PAIR], bf16, name=f"y1sb{P}", tag="y1sb")
                yq = [
                    yqp.tile([128, 512], f32, name=f"y1q{P}_{q}", tag="yq")
                    for q in range(4)
                ]
                st[P]["y1sb"] = y1sb
                st[P]["yq"] = yq

            def s3_contract(P, glo, ghi):
                hxs, yq = st[P]["hxs"], st[P]["yq"]
                for g in range(glo, ghi):
                    nc.tensor.ldweights(w1s[:, g, :])
                    for q in range(4):
                        mm_nold(
                            yq[q][:], w1s[:, g, :],
                            hxs[g // 2][:, g % 2, 512 * q : 512 * (q + 1)],
                            start=(g == 0), stop=(g == G1 - 1),
                        )

            def s3_fini(P):
                y1sb, yq = st[P]["y1sb"], st[P]["yq"]
                for q in range(4):
                    nc.scalar.activation(
                        y1sb[:, 512 * q : 512 * (q + 1)], yq[q][:], Relu, bias=b1s[:]
                    )
                w = D // 2
                while w >= 1:
                    a = y1sb[:].rearrange("p (b d) -> p b d", d=D)
                    dst = (
                        oacc1[:, P * SPP : (P + 1) * SPP] if w == 1 else a[:, :, 0:w]
                    )
                    nc.vector.tensor_add(dst, a[:, :, 0:w], a[:, :, w : 2 * w])
                    w //= 2
                del st[P]

            for P in range(NPAIR + 2):
                p1, p2, p3 = P, P - 1, P - 2
                if p1 < NPAIR:
                    s1_load(p1)
                    s1_build(p1, 0)
                if 0 <= p2 < NPAIR:
                    s2(p2)
                if p1 < NPAIR:
                    s1_build(p1, 1)
                if 0 <= p3 < NPAIR:
                    s3_muls(p3)
                    s3_contract(p3, 0, 4)
                if p1 < NPAIR:
                    s1_build(p1, 2)
                if 0 <= p3 < NPAIR:
                    s3_contract(p3, 4, 8)
                if p1 < NPAIR:
                    s1_build(p1, 3)
                if 0 <= p3 < NPAIR:
                    s3_contract(p3, 8, 12)
                if p1 < NPAIR:
                    s1_build(p1, 4)
                if 0 <= p3 < NPAIR:
                    s3_contract(p3, 12, 16)
                    s3_fini(p3)

            nc.gpsimd.dma_start(out=out0[:], in_=oacc0[H1:O, :])
            nc.gpsimd.dma_start(out=out1[:], in_=oacc1[:])

    nc.finalize()
    return nc


def _get_nc():
    if "nc" not in _CACHE:
        _CACHE["nc"] = _build_nc()
    return _CACHE["nc"]


def _l0_pairs():
    return [(h, f) for h in range(F) for f in range(h + 1, F)]


def make_l0(w0_np):
    """A0 build matrix [32, 640] and folded weights [640, 128] (zero-padded)."""
    pairs = _l0_pairs()
    A0 = np.zeros((F, NG0 * 128), np.float32)
    w0fold = np.zeros((NG0 * 128, O), np.float32)
    for k, (h, f) in enumerate(pairs):
        A0[h, k] = 1.0
        A0[f, k] = 1.0
        w0fold[k] = (w0_np[:, h * F + f] + w0_np[:, f * F + h]) / 2
    for h in range(F):
        k = 496 + h
        A0[h, k] = 1.0
        c = w0_np[:, h * F + h].copy()
        for f in range(F):
            if f != h:
                c -= 0.5 * (w0_np[:, h * F + f] + w0_np[:, f * F + h])
        w0fold[k] = c
    return A0, w0fold


def kernel(cin_inputs, w0, b0, w1, b1, _trace=False):
    from concourse.bass_utils import run_bass_kernel_spmd

    x = np.asarray(cin_inputs, dtype=np.float32)
    assert x.shape == (B_FULL, F, D)
    bf = ml_dtypes.bfloat16
    xt_all = np.ascontiguousarray(
        x.reshape(N_CORES, BS, F, D).transpose(0, 2, 1, 3)
    ).astype(bf).reshape(N_CORES, F, BS * D)
    xt_all = np.ascontiguousarray(np.tile(xt_all, (1, 4, 1)))

    w0_np = np.asarray(w0, dtype=np.float32)
    A0, w0fold = make_l0(w0_np)
    a0c = np.ascontiguousarray(
        np.tile(A0.reshape(F, NG0, 128), (4, 1, 1))
    ).astype(bf)
    w0fc = np.ascontiguousarray(
        w0fold.reshape(NG0, 128, O).transpose(1, 0, 2)
    ).astype(bf)
    w1tc = np.ascontiguousarray(np.asarray(w1, dtype=np.float32).T).astype(bf)
    b0c = np.asarray(b0, dtype=np.float32).reshape(O, 1).copy()
    b1c = np.asarray(b1, dtype=np.float32).reshape(O, 1).copy()

    nc = _get_nc()
    in_maps = []
    for i in range(N_CORES):
        in_maps.append(
            {
                "xt": xt_all[i],
                "a0": a0c, "w0f": w0fc, "w1t": w1tc,
                "b0": b0c, "b1": b1c,
            }
        )
    res = run_bass_kernel_spmd(nc, in_maps, core_ids=list(range(N_CORES)), trace=_trace)
    outs = []
    for r in res.results:
        o = np.concatenate([r["out0"], r["out1"]], axis=0).T
        outs.append(o)
    full = np.concatenate(outs, axis=0).astype(np.float32)
    if _trace:
        return full, res
    return full
